# revision 6
# baseline (speedup 1.0000x reference)
"""DeepseekV3 MoE kernel for 8 Trainium2 NeuronCores (expert-parallel).

Strategy:
  - Host: grouped top-k gating (exact replica of the reference jax ops, on CPU),
    token dispatch (gather tokens per expert, zero-padded to fixed capacity).
  - Device (SPMD over 8 cores): core c owns routed experts 8c..8c+7 and a
    64-wide slice of the intermediate dim of both shared experts.
    All matmuls run as float32r (full-rate fp32 on the PE array).
    Weights are shipped in partition-major layout for large DMA packets.
  - Host: scatter-add expert outputs back by token, sum shared partials.

Shapes (hardcoded): T=1024, H=1024, I=512, E=64, S=2, G=8, TOPK_GROUP=4, K=8.
"""
import numpy as np
from contextlib import ExitStack

import concourse.bass as bass
from concourse import mybir, tile, bacc
from concourse.bass_utils import run_bass_kernel_spmd

f32 = mybir.dt.float32
f32r = mybir.dt.float32r
AF = mybir.ActivationFunctionType

T, H, I, E, S = 1024, 1024, 512, 64, 2
G, TOPK_GROUP, K = 8, 4, 8
I2 = 2 * I
N_CORES = 8
E_LOC = E // N_CORES          # 8 experts per core
C = 256                       # per-expert token capacity (mean load is 128)
C_OUT = 160                   # rows of y actually returned (rest: host fallback)
HT = H // 128                 # 8 k-tiles over hidden dim
IT = I // 128                 # 4 tiles over intermediate dim
ISH = I // N_CORES            # 64-wide shared-expert slice per core

_TRACE = False
_CACHED_NC = None
LAST_RESULTS = None


def _build_nc():
    nc = bacc.Bacc("TRN2", target_bir_lowering=False, debug=False)

    # partition-major layouts: [..., 128, chunk, free] so each partition's
    # DRAM run is contiguous (big DMA packets)
    xgt_d = nc.dram_tensor("xgt", [E_LOC, 2, 128, HT // 2, C], f32r,
                           kind="ExternalInput")
    wgu_d = nc.dram_tensor("wgu", [E_LOC, 4, 128, 2, I2], f32r,
                           kind="ExternalInput")
    wd_d = nc.dram_tensor("wd", [E_LOC, 2, 128, 2, H], f32r, kind="ExternalInput")
    cwp_d = nc.dram_tensor("cwp", [128, E_LOC * 2], f32, kind="ExternalInput")
    xt_d = nc.dram_tensor("xt", [2, 128, HT // 2, T], f32r, kind="ExternalInput")
    swh_d = nc.dram_tensor("swh", [S, 128, HT, 2 * ISH], f32r, kind="ExternalInput")
    sdc_d = nc.dram_tensor("sdc", [S * ISH, H], f32r, kind="ExternalInput")
    y_d = nc.dram_tensor("y", [E_LOC, C_OUT, H], f32, kind="ExternalOutput")
    sh_d = nc.dram_tensor("sh", [T, H], f32, kind="ExternalOutput")

    with tile.TileContext(nc) as tc, ExitStack() as ctx:
        wgu_p = ctx.enter_context(tc.tile_pool(name="wgu", bufs=2))
        wd_p = ctx.enter_context(tc.tile_pool(name="wd", bufs=2))
        xgt_p = ctx.enter_context(tc.tile_pool(name="xgt", bufs=3))
        h_p = ctx.enter_context(tc.tile_pool(name="h", bufs=2))
        y_p = ctx.enter_context(tc.tile_pool(name="y", bufs=3))
        const_p = ctx.enter_context(tc.tile_pool(name="const", bufs=1))
        shh_p = ctx.enter_context(tc.tile_pool(name="shh", bufs=2))
        psA = ctx.enter_context(tc.tile_pool(name="psA", bufs=4, space="PSUM"))
        psB = ctx.enter_context(tc.tile_pool(name="psB", bufs=4, space="PSUM"))

        cw_sb = const_p.tile([128, E_LOC * 2], f32, tag="cw")

        def emit_expert(e):
            if e == 0:
                pass
            xg_ch = []
            for q in range(2):
                xg = xgt_p.tile([128, HT // 2, C], f32r, tag=f"xgt{q}")
                nc.sync.dma_start(xg[:], xgt_d.ap()[e, q])
                xg_ch.append(xg)
            wgu_ch = []
            for ch in range(4):
                wg = wgu_p.tile([128, 2, I2], f32r, tag=f"wguc{ch}")
                nc.sync.dma_start(wg[:], wgu_d.ap()[e, ch])
                wgu_ch.append(wg)

            h_t = []
            for it in range(IT):
                pss = []
                for half in range(2):  # gate, up
                    ps = psA.tile([128, C], f32, tag="gu")
                    off = half * I + it * 128
                    for h in range(HT):
                        nc.tensor.matmul(
                            ps[:], wgu_ch[h // 2][:, h % 2, off:off + 128],
                            xg_ch[h // 4][:, h % 4, :],
                            start=(h == 0), stop=(h == HT - 1))
                    pss.append(ps)
                sl = h_p.tile([128, C], f32, tag="silu")
                nc.scalar.activation(sl[:], pss[0][:], AF.Silu)
                hh = h_p.tile([128, C], f32r, tag=f"h{it}")
                nc.vector.tensor_mul(hh[:], sl[:], pss[1][:])
                h_t.append(hh)

            wd_ch = []
            for q in range(2):
                wd = wd_p.tile([128, 2, H], f32r, tag=f"wd{q}")
                nc.sync.dma_start(wd[:], wd_d.ap()[e, q])
                wd_ch.append(wd)
            for ct in range(C // 128):
                rows = min(128, C_OUT - ct * 128)
                yo = y_p.tile([128, H], f32, tag="y")
                for nt in range(H // 512):
                    ps_y = psB.tile([128, 512], f32, tag="big")
                    for it in range(IT):
                        nc.tensor.matmul(ps_y[:], h_t[it][:, ct * 128:(ct + 1) * 128],
                                         wd_ch[it // 2][:, it % 2, nt * 512:(nt + 1) * 512],
                                         start=(it == 0), stop=(it == IT - 1))
                    nc.vector.tensor_scalar_mul(
                        yo[:, nt * 512:(nt + 1) * 512], ps_y[:],
                        cw_sb[:, e * 2 + ct:e * 2 + ct + 1])
                seng = nc.scalar if (e * 2 + ct) % 2 == 0 else nc.gpsimd
                seng.dma_start(y_d.ap()[e, ct * 128:ct * 128 + rows, :],
                               yo[:rows, :])

        hc_t = []

        def emit_shared_gu():
            xt_ch = []
            for q in range(2):
                xt = const_p.tile([128, HT // 2, T], f32r, tag=f"xt{q}")
                nc.sync.dma_start(xt[:], xt_d.ap()[q])
                xt_ch.append(xt)
            swh_sb = []
            for s in range(S):
                sw = const_p.tile([128, HT, 2 * ISH], f32r, tag=f"swh{s}")
                nc.sync.dma_start(sw[:], swh_d.ap()[s])
                swh_sb.append(sw)
            sdc_sb = const_p.tile([S * ISH, H], f32r, tag="sdc")
            nc.sync.dma_start(sdc_sb[:], sdc_d.ap()[:])

            for tt in range(2):  # two 512-token halves
                hc = shh_p.tile([128, 512], f32r, tag=f"hc{tt}")
                for s in range(S):
                    ps = psB.tile([128, 512], f32, tag="big")
                    for h in range(HT):
                        nc.tensor.matmul(ps[:], swh_sb[s][:, h, :],
                                         xt_ch[h // 4][:, h % 4, tt * 512:(tt + 1) * 512],
                                         start=(h == 0), stop=(h == HT - 1))
                    sl = shh_p.tile([ISH, 512], f32, tag="slsh")
                    nc.scalar.activation(sl[:], ps[0:ISH, :], AF.Silu)
                    nc.vector.tensor_mul(hc[s * ISH:(s + 1) * ISH, :], sl[:],
                                         ps[ISH:2 * ISH, :])
                hc_t.append(hc)
            return sdc_sb

        def emit_shared_down(sdc_sb):
            for tt in range(2):
                hc = hc_t[tt]
                for tp in range(4):
                    so = y_p.tile([128, H], f32, tag="y")
                    for hh2 in range(2):
                        ps2 = psB.tile([128, 512], f32, tag="big")
                        nc.tensor.matmul(ps2[:], hc[:, tp * 128:(tp + 1) * 128],
                                         sdc_sb[:, hh2 * 512:(hh2 + 1) * 512],
                                         start=True, stop=True)
                        nc.vector.tensor_copy(so[:, hh2 * 512:(hh2 + 1) * 512],
                                              ps2[:])
                    seng = nc.scalar if tp % 2 == 0 else nc.gpsimd
                    seng.dma_start(
                        sh_d.ap()[(tt * 4 + tp) * 128:(tt * 4 + tp + 1) * 128, :],
                        so[:])

        emit_expert(0)
        nc.sync.dma_start(cw_sb[:], cwp_d.ap()[:])
        for e in range(1, 6):
            emit_expert(e)
        sdc_sb = emit_shared_gu()
        for e in range(6, E_LOC):
            emit_expert(e)
        emit_shared_down(sdc_sb)
    nc.compile()
    return nc


def _route(x, gate_w):
    """Exact replica of the reference's grouped top-k gating, on CPU jax."""
    import jax
    import jax.numpy as jnp
    cpu = jax.devices("cpu")[0]
    with jax.default_device(cpu):
        xj = jax.device_put(np.asarray(x), cpu)
        gj = jax.device_put(np.asarray(gate_w), cpu)
        logits = xj @ gj.T
        t = logits.shape[0]
        group_size = E // G
        group_logits = logits.reshape(t, G, group_size)
        gw, gi = jax.lax.top_k(group_logits, TOPK_GROUP)
        gw = gw.reshape(t, G * TOPK_GROUP)
        gi = gi.reshape(t, G * TOPK_GROUP)
        topk_w, ti = jax.lax.top_k(gw, K)
        sel_group = ti // TOPK_GROUP
        expert_in_group = jnp.take_along_axis(gi, ti, axis=1)
        topk_idx = sel_group * group_size + expert_in_group
        topk_w = topk_w / (topk_w.sum(axis=-1, keepdims=True) + 1e-20)
    return np.asarray(topk_idx), np.asarray(topk_w).astype(np.float32)


def _expert_np(xrows, w_gu_e, w_d_e):
    """Reference expert math in numpy fp32 (overflow fallback only)."""
    g = xrows @ w_gu_e
    a = g[:, :I]
    hidden = (a / (1.0 + np.exp(-a))) * g[:, I:]
    return hidden @ w_d_e


def kernel(x, gate_w, w_gu, w_d, s_gu, s_d):
    global _CACHED_NC, LAST_RESULTS
    x = np.ascontiguousarray(np.asarray(x, dtype=np.float32))
    gate_w = np.ascontiguousarray(np.asarray(gate_w, dtype=np.float32))
    w_gu = np.ascontiguousarray(np.asarray(w_gu, dtype=np.float32))
    w_d = np.ascontiguousarray(np.asarray(w_d, dtype=np.float32))
    s_gu = np.ascontiguousarray(np.asarray(s_gu, dtype=np.float32))
    s_d = np.ascontiguousarray(np.asarray(s_d, dtype=np.float32))

    topk_idx, topk_w = _route(x, gate_w)

    flat_e = topk_idx.ravel()
    flat_t = np.repeat(np.arange(T), K)
    flat_w = topk_w.ravel()
    order = np.argsort(flat_e, kind="stable")
    sorted_t = flat_t[order]
    sorted_w = flat_w[order]
    counts = np.bincount(flat_e, minlength=E)
    starts = np.zeros(E + 1, np.int64)
    np.cumsum(counts, out=starts[1:])

    xT = np.ascontiguousarray(x.T)  # [H, T]
    xgt = np.zeros((E, H, C), np.float32)
    cw = np.zeros((E, C), np.float32)
    overflow = []
    for e in range(E):
        n = int(counts[e])
        toks = sorted_t[starts[e]:starts[e] + n]
        ws = sorted_w[starts[e]:starts[e] + n]
        nn = min(n, C)
        xgt[e, :, :nn] = xT[:, toks[:nn]]
        cw[e, :nn] = ws[:nn]
        if n > C_OUT:
            keep = min(n, C_OUT)
            overflow.append((e, toks[keep:], ws[keep:]))

    # partition-major shuffles for big DMA packets
    xgt_s = np.ascontiguousarray(
        xgt.reshape(E, 2, HT // 2, 128, C).transpose(0, 1, 3, 2, 4))
    wgu_s = np.ascontiguousarray(
        w_gu.reshape(E, 4, 2, 128, I2).transpose(0, 1, 3, 2, 4))
    wd_s = np.ascontiguousarray(
        w_d.reshape(E, 2, 2, 128, H).transpose(0, 1, 3, 2, 4))
    xt_s = np.ascontiguousarray(
        xT.reshape(2, HT // 2, 128, T).transpose(0, 2, 1, 3))      # [2,128,4,T]

    # packed combine weights: cwp[p, e*2+j] = cw[e, j*128+p]
    cwp = np.ascontiguousarray(
        cw.reshape(E, 2, 128).transpose(2, 0, 1).reshape(128, E * 2))

    if _CACHED_NC is None:
        _CACHED_NC = _build_nc()
    nc = _CACHED_NC

    in_maps = []
    for c in range(N_CORES):
        lo = c * E_LOC
        sl = slice(c * ISH, (c + 1) * ISH)
        swh = np.concatenate([s_gu[:, :, sl], s_gu[:, :, I:][:, :, sl]], axis=2)
        swh_s = np.ascontiguousarray(
            swh.reshape(S, HT, 128, 2 * ISH).transpose(0, 2, 1, 3))
        sdc = np.ascontiguousarray(s_d[:, sl, :].reshape(S * ISH, H))
        in_maps.append({
            "xgt": xgt_s[lo:lo + E_LOC],
            "wgu": wgu_s[lo:lo + E_LOC],
            "wd": wd_s[lo:lo + E_LOC],
            "cwp": np.ascontiguousarray(cwp[:, lo * 2:(lo + E_LOC) * 2]),
            "xt": xt_s,
            "swh": swh_s,
            "sdc": sdc,
        })

    res = run_bass_kernel_spmd(nc, in_maps, list(range(N_CORES)), trace=_TRACE)
    LAST_RESULTS = res

    out = np.zeros((T, H), np.float32)
    for c in range(N_CORES):
        out += res.results[c]["sh"]

    y_all = np.concatenate([res.results[c]["y"] for c in range(N_CORES)], axis=0)
    routed_rows = np.empty((T * K, H), np.float32)
    pos = 0
    tok_order = np.empty(T * K, np.int64)
    for e in range(E):
        n = min(int(counts[e]), C_OUT)
        routed_rows[pos:pos + n] = y_all[e, :n]
        tok_order[pos:pos + n] = sorted_t[starts[e]:starts[e] + n]
        pos += n
    inv = np.argsort(tok_order[:pos], kind="stable")
    if pos == T * K:
        routed = routed_rows[inv].reshape(T, K, H).sum(axis=1)
        out += routed
    else:
        np.add.at(out, tok_order[:pos][inv], routed_rows[:pos][inv])

    for e, toks, ws in overflow:
        y_extra = _expert_np(x[toks], w_gu[e], w_d[e]) * ws[:, None]
        np.add.at(out, toks, y_extra)

    return out


# revision 7
# speedup vs baseline: 1.0038x; 1.0038x over previous
"""DeepseekV3 MoE kernel for 8 Trainium2 NeuronCores (expert-parallel).

Strategy:
  - Host: grouped top-k gating (exact replica of the reference jax ops, on CPU),
    token dispatch (gather tokens per expert, zero-padded to fixed capacity).
  - Device (SPMD over 8 cores): core c owns routed experts 8c..8c+7 and a
    64-wide slice of the intermediate dim of both shared experts.
    All matmuls run as float32r (full-rate fp32 on the PE array).
    Weights are shipped in partition-major layout for large DMA packets.
  - Host: scatter-add expert outputs back by token, sum shared partials.

Shapes (hardcoded): T=1024, H=1024, I=512, E=64, S=2, G=8, TOPK_GROUP=4, K=8.
"""
import numpy as np
from contextlib import ExitStack

import concourse.bass as bass
from concourse import mybir, tile, bacc
from concourse.bass_utils import run_bass_kernel_spmd

f32 = mybir.dt.float32
f32r = mybir.dt.float32r
AF = mybir.ActivationFunctionType

T, H, I, E, S = 1024, 1024, 512, 64, 2
G, TOPK_GROUP, K = 8, 4, 8
I2 = 2 * I
N_CORES = 8
E_LOC = E // N_CORES          # 8 experts per core
C = 256                       # per-expert token capacity (mean load is 128)
C_OUT = 160                   # rows of y actually returned (rest: host fallback)
HT = H // 128                 # 8 k-tiles over hidden dim
IT = I // 128                 # 4 tiles over intermediate dim
ISH = I // N_CORES            # 64-wide shared-expert slice per core

_TRACE = False
_CACHED_NC = None
LAST_RESULTS = None


def _build_nc():
    nc = bacc.Bacc("TRN2", target_bir_lowering=False, debug=False)

    # partition-major layouts: [..., 128, chunk, free] so each partition's
    # DRAM run is contiguous (big DMA packets)
    xgt_d = nc.dram_tensor("xgt", [E_LOC, 2, 128, HT // 2, C], f32r,
                           kind="ExternalInput")
    wgu_d = nc.dram_tensor("wgu", [E_LOC, 4, 128, 2, I2], f32r,
                           kind="ExternalInput")
    wd_d = nc.dram_tensor("wd", [E_LOC, 2, 128, 2, H], f32r, kind="ExternalInput")
    cwp_d = nc.dram_tensor("cwp", [128, E_LOC * 2], f32, kind="ExternalInput")
    xt_d = nc.dram_tensor("xt", [2, 128, HT // 2, T], f32r, kind="ExternalInput")
    swh_d = nc.dram_tensor("swh", [S, 128, HT, 2 * ISH], f32r, kind="ExternalInput")
    sdc_d = nc.dram_tensor("sdc", [S * ISH, H], f32r, kind="ExternalInput")
    y_d = nc.dram_tensor("y", [E_LOC, C_OUT, H], f32, kind="ExternalOutput")
    sh_d = nc.dram_tensor("sh", [T, H], f32, kind="ExternalOutput")

    with tile.TileContext(nc) as tc, ExitStack() as ctx:
        wgu_p = ctx.enter_context(tc.tile_pool(name="wgu", bufs=2))
        wd_p = ctx.enter_context(tc.tile_pool(name="wd", bufs=2))
        xgt_p = ctx.enter_context(tc.tile_pool(name="xgt", bufs=3))
        h_p = ctx.enter_context(tc.tile_pool(name="h", bufs=2))
        y_p = ctx.enter_context(tc.tile_pool(name="y", bufs=3))
        const_p = ctx.enter_context(tc.tile_pool(name="const", bufs=1))
        shh_p = ctx.enter_context(tc.tile_pool(name="shh", bufs=2))
        psA = ctx.enter_context(tc.tile_pool(name="psA", bufs=4, space="PSUM"))
        psB = ctx.enter_context(tc.tile_pool(name="psB", bufs=4, space="PSUM"))

        cw_sb = const_p.tile([128, E_LOC * 2], f32, tag="cw")

        def emit_expert(e):
            if e == 0:
                pass
            xg_ch = []
            for q in range(2):
                xg = xgt_p.tile([128, HT // 2, C], f32r, tag=f"xgt{q}")
                nc.sync.dma_start(xg[:], xgt_d.ap()[e, q])
                xg_ch.append(xg)
            wgu_ch = []
            for ch in range(4):
                wg = wgu_p.tile([128, 2, I2], f32r, tag=f"wguc{ch}")
                nc.sync.dma_start(wg[:], wgu_d.ap()[e, ch])
                wgu_ch.append(wg)

            h_t = []
            for it in range(IT):
                pss = []
                for half in range(2):  # gate, up
                    ps = psA.tile([128, C], f32, tag="gu")
                    off = half * I + it * 128
                    for h in range(HT):
                        nc.tensor.matmul(
                            ps[:], wgu_ch[h // 2][:, h % 2, off:off + 128],
                            xg_ch[h // 4][:, h % 4, :],
                            start=(h == 0), stop=(h == HT - 1))
                    pss.append(ps)
                sl = h_p.tile([128, C], f32, tag="silu")
                nc.scalar.activation(sl[:], pss[0][:], AF.Silu)
                hh = h_p.tile([128, C], f32r, tag=f"h{it}")
                nc.vector.tensor_mul(hh[:], sl[:], pss[1][:])
                h_t.append(hh)

            wd_ch = []
            for q in range(2):
                wd = wd_p.tile([128, 2, H], f32r, tag=f"wd{q}")
                nc.sync.dma_start(wd[:], wd_d.ap()[e, q])
                wd_ch.append(wd)
            for ct in range(C // 128):
                rows = min(128, C_OUT - ct * 128)
                yo = y_p.tile([128, H], f32, tag="y")
                for nt in range(H // 512):
                    ps_y = psB.tile([128, 512], f32, tag="big")
                    for it in range(IT):
                        nc.tensor.matmul(ps_y[:], h_t[it][:, ct * 128:(ct + 1) * 128],
                                         wd_ch[it // 2][:, it % 2, nt * 512:(nt + 1) * 512],
                                         start=(it == 0), stop=(it == IT - 1))
                    nc.vector.tensor_scalar_mul(
                        yo[:, nt * 512:(nt + 1) * 512], ps_y[:],
                        cw_sb[:, e * 2 + ct:e * 2 + ct + 1])
                seng = nc.scalar if (e * 2 + ct) % 2 == 0 else nc.gpsimd
                seng.dma_start(y_d.ap()[e, ct * 128:ct * 128 + rows, :],
                               yo[:rows, :])

        hc_t = []

        def emit_shared_gu():
            xt_ch = []
            for q in range(2):
                xt = const_p.tile([128, HT // 2, T], f32r, tag=f"xt{q}")
                nc.sync.dma_start(xt[:], xt_d.ap()[q])
                xt_ch.append(xt)
            swh_sb = []
            for s in range(S):
                sw = const_p.tile([128, HT, 2 * ISH], f32r, tag=f"swh{s}")
                nc.sync.dma_start(sw[:], swh_d.ap()[s])
                swh_sb.append(sw)
            sdc_sb = const_p.tile([S * ISH, H], f32r, tag="sdc")
            nc.sync.dma_start(sdc_sb[:], sdc_d.ap()[:])

            for tt in range(2):  # two 512-token halves
                hc = shh_p.tile([128, 512], f32r, tag=f"hc{tt}")
                for s in range(S):
                    ps = psB.tile([128, 512], f32, tag="big")
                    for h in range(HT):
                        nc.tensor.matmul(ps[:], swh_sb[s][:, h, :],
                                         xt_ch[h // 4][:, h % 4, tt * 512:(tt + 1) * 512],
                                         start=(h == 0), stop=(h == HT - 1))
                    sl = shh_p.tile([ISH, 512], f32, tag="slsh")
                    nc.scalar.activation(sl[:], ps[0:ISH, :], AF.Silu)
                    nc.vector.tensor_mul(hc[s * ISH:(s + 1) * ISH, :], sl[:],
                                         ps[ISH:2 * ISH, :])
                hc_t.append(hc)
            return sdc_sb

        def emit_shared_down(sdc_sb):
            for tt in range(2):
                hc = hc_t[tt]
                for tp in range(4):
                    so = y_p.tile([128, H], f32, tag="y")
                    for hh2 in range(2):
                        ps2 = psB.tile([128, 512], f32, tag="big")
                        nc.tensor.matmul(ps2[:], hc[:, tp * 128:(tp + 1) * 128],
                                         sdc_sb[:, hh2 * 512:(hh2 + 1) * 512],
                                         start=True, stop=True)
                        nc.vector.tensor_copy(so[:, hh2 * 512:(hh2 + 1) * 512],
                                              ps2[:])
                    seng = nc.scalar if tp % 2 == 0 else nc.gpsimd
                    seng.dma_start(
                        sh_d.ap()[(tt * 4 + tp) * 128:(tt * 4 + tp + 1) * 128, :],
                        so[:])

        nc.sync.dma_start(cw_sb[:], cwp_d.ap()[:])
        for e in range(6):
            emit_expert(e)
        sdc_sb = emit_shared_gu()
        emit_shared_down(sdc_sb)
        for e in range(6, E_LOC):
            emit_expert(e)
    nc.compile()
    return nc


def _route(x, gate_w):
    """Exact replica of the reference's grouped top-k gating, on CPU jax."""
    import jax
    import jax.numpy as jnp
    cpu = jax.devices("cpu")[0]
    with jax.default_device(cpu):
        xj = jax.device_put(np.asarray(x), cpu)
        gj = jax.device_put(np.asarray(gate_w), cpu)
        logits = xj @ gj.T
        t = logits.shape[0]
        group_size = E // G
        group_logits = logits.reshape(t, G, group_size)
        gw, gi = jax.lax.top_k(group_logits, TOPK_GROUP)
        gw = gw.reshape(t, G * TOPK_GROUP)
        gi = gi.reshape(t, G * TOPK_GROUP)
        topk_w, ti = jax.lax.top_k(gw, K)
        sel_group = ti // TOPK_GROUP
        expert_in_group = jnp.take_along_axis(gi, ti, axis=1)
        topk_idx = sel_group * group_size + expert_in_group
        topk_w = topk_w / (topk_w.sum(axis=-1, keepdims=True) + 1e-20)
    return np.asarray(topk_idx), np.asarray(topk_w).astype(np.float32)


def _expert_np(xrows, w_gu_e, w_d_e):
    """Reference expert math in numpy fp32 (overflow fallback only)."""
    g = xrows @ w_gu_e
    a = g[:, :I]
    hidden = (a / (1.0 + np.exp(-a))) * g[:, I:]
    return hidden @ w_d_e


def kernel(x, gate_w, w_gu, w_d, s_gu, s_d):
    global _CACHED_NC, LAST_RESULTS
    x = np.ascontiguousarray(np.asarray(x, dtype=np.float32))
    gate_w = np.ascontiguousarray(np.asarray(gate_w, dtype=np.float32))
    w_gu = np.ascontiguousarray(np.asarray(w_gu, dtype=np.float32))
    w_d = np.ascontiguousarray(np.asarray(w_d, dtype=np.float32))
    s_gu = np.ascontiguousarray(np.asarray(s_gu, dtype=np.float32))
    s_d = np.ascontiguousarray(np.asarray(s_d, dtype=np.float32))

    topk_idx, topk_w = _route(x, gate_w)

    flat_e = topk_idx.ravel()
    flat_t = np.repeat(np.arange(T), K)
    flat_w = topk_w.ravel()
    order = np.argsort(flat_e, kind="stable")
    sorted_t = flat_t[order]
    sorted_w = flat_w[order]
    counts = np.bincount(flat_e, minlength=E)
    starts = np.zeros(E + 1, np.int64)
    np.cumsum(counts, out=starts[1:])

    xT = np.ascontiguousarray(x.T)  # [H, T]
    xgt = np.zeros((E, H, C), np.float32)
    cw = np.zeros((E, C), np.float32)
    overflow = []
    for e in range(E):
        n = int(counts[e])
        toks = sorted_t[starts[e]:starts[e] + n]
        ws = sorted_w[starts[e]:starts[e] + n]
        nn = min(n, C)
        xgt[e, :, :nn] = xT[:, toks[:nn]]
        cw[e, :nn] = ws[:nn]
        if n > C_OUT:
            keep = min(n, C_OUT)
            overflow.append((e, toks[keep:], ws[keep:]))

    # partition-major shuffles for big DMA packets
    xgt_s = np.ascontiguousarray(
        xgt.reshape(E, 2, HT // 2, 128, C).transpose(0, 1, 3, 2, 4))
    wgu_s = np.ascontiguousarray(
        w_gu.reshape(E, 4, 2, 128, I2).transpose(0, 1, 3, 2, 4))
    wd_s = np.ascontiguousarray(
        w_d.reshape(E, 2, 2, 128, H).transpose(0, 1, 3, 2, 4))
    xt_s = np.ascontiguousarray(
        xT.reshape(2, HT // 2, 128, T).transpose(0, 2, 1, 3))      # [2,128,4,T]

    # packed combine weights: cwp[p, e*2+j] = cw[e, j*128+p]
    cwp = np.ascontiguousarray(
        cw.reshape(E, 2, 128).transpose(2, 0, 1).reshape(128, E * 2))

    if _CACHED_NC is None:
        _CACHED_NC = _build_nc()
    nc = _CACHED_NC

    in_maps = []
    for c in range(N_CORES):
        lo = c * E_LOC
        sl = slice(c * ISH, (c + 1) * ISH)
        swh = np.concatenate([s_gu[:, :, sl], s_gu[:, :, I:][:, :, sl]], axis=2)
        swh_s = np.ascontiguousarray(
            swh.reshape(S, HT, 128, 2 * ISH).transpose(0, 2, 1, 3))
        sdc = np.ascontiguousarray(s_d[:, sl, :].reshape(S * ISH, H))
        in_maps.append({
            "xgt": xgt_s[lo:lo + E_LOC],
            "wgu": wgu_s[lo:lo + E_LOC],
            "wd": wd_s[lo:lo + E_LOC],
            "cwp": np.ascontiguousarray(cwp[:, lo * 2:(lo + E_LOC) * 2]),
            "xt": xt_s,
            "swh": swh_s,
            "sdc": sdc,
        })

    res = run_bass_kernel_spmd(nc, in_maps, list(range(N_CORES)), trace=_TRACE)
    LAST_RESULTS = res

    out = np.zeros((T, H), np.float32)
    for c in range(N_CORES):
        out += res.results[c]["sh"]

    y_all = np.concatenate([res.results[c]["y"] for c in range(N_CORES)], axis=0)
    routed_rows = np.empty((T * K, H), np.float32)
    pos = 0
    tok_order = np.empty(T * K, np.int64)
    for e in range(E):
        n = min(int(counts[e]), C_OUT)
        routed_rows[pos:pos + n] = y_all[e, :n]
        tok_order[pos:pos + n] = sorted_t[starts[e]:starts[e] + n]
        pos += n
    inv = np.argsort(tok_order[:pos], kind="stable")
    if pos == T * K:
        routed = routed_rows[inv].reshape(T, K, H).sum(axis=1)
        out += routed
    else:
        np.add.at(out, tok_order[:pos][inv], routed_rows[:pos][inv])

    for e, toks, ws in overflow:
        y_extra = _expert_np(x[toks], w_gu[e], w_d[e]) * ws[:, None]
        np.add.at(out, toks, y_extra)

    return out
